# revision 95
# baseline (speedup 1.0000x reference)
"""Trainium2 Bass kernel: LocalCausalTransformerBlock (window-3 causal attention).

Sharding: 8-way sequence-parallel. B=2 x N=2048 = 4096 tokens -> 8 chunks of
512 tokens (4 chunks per batch row). Each core gets its 512 tokens plus a
2-token halo (the preceding tokens of the same sequence, prepended host-side)
so the window-3 causal attention needs no cross-core communication. Weights
are replicated.

Everything on-device is channel-major (channels on partitions, tokens on the
free axis): the host hands x pre-transposed with the halo prepended and
un-transposes the output, so the PE does only matmuls - no transposes at all.
Both layernorms run channel-major: pairwise chunk folds + a gpsimd partition
all-reduce produce per-token sums broadcast to every partition, the rstd/mu
row math runs on those broadcast tiles in bf16, and the apply is two
tensor-tensor passes (in two token halves so downstream consumers start
sooner). LN gammas fold into the following matmul's weights host-side.

The four big matmuls (qkv/proj/fc1/fc2) run in fp8e4m3 with DoubleRow perf
mode (0.5 cycles/row). Weights are pre-scaled per output column to a power of
two near absmax~2; the descale rides the evictions. The proj and fc2
evictions are scalar_tensor_tensor ops that fuse descale + residual add in
one instruction (nonzero proj/fc2 biases, if ever present, ride an extra
ones-chunk in the contraction). k/v channels are host-permuted head-minor
(head = partition // 8, identical in every chunk) so the softmax probs
broadcast from 16 head rows to 128 partitions is a single stride-0 DMA via a
DRAM round-trip. Softmax needs no max-subtraction (window-3 scores are
small). DMAs are spread over the SP/Activation/Pool queues, which the cost
model executes concurrently.
"""

import sys

for _p in ("/opt/trn_rl_repo",):
    if _p not in sys.path:
        sys.path.insert(0, _p)

import numpy as np
import ml_dtypes

P = 128
D = 1024
H = 16
HD = 64
H3 = 3 * D
HID = 4096
T = 512            # real tokens per core
TH = T + 2         # token axis with 2-token halo (halo stored first)
NCORE = 8
EPS = 1e-5
BF = ml_dtypes.bfloat16
F8 = ml_dtypes.float8_e4m3

# which weights carry the fp8 quantization residual (2x k-chunks)
COMP = {"qkv": False, "proj": False, "fc1": False, "fc2": False}

# packed f32 const columns
_C = {}
_off = 0
for _name, _w in [("qkvb", 24), ("qkvs", 24), ("projs", 8),
                  ("fc1b", 32), ("fc1s", 32), ("fc2s", 8),
                  ("khs", 32), ("khb", 32)]:
    _C[_name] = _off
    _off += _w
CPAK_W = _off
# packed bf16 const columns: hmask, emk
_B = {"hmask": 0, "emk": 128, "em2": 131}
BPAK_W = 259

_CACHE: dict = {}


def _build_program(bias_p=False, bias_f=False):
    """bias_p/bias_f: include ones-chunks in proj/fc2 matmuls to add a
    nonzero proj_b/fc2_b (the fused residual evictions have no other slot
    for them). Left off when the biases are zero."""
    import concourse.bass as bass
    import concourse.tile as tile
    import concourse.ap as cap
    from concourse import bacc, mybir, bass_isa
    from contextlib import ExitStack

    f32 = mybir.dt.float32
    bf16 = mybir.dt.bfloat16
    fp8 = mybir.dt.float8e4
    ALU = mybir.AluOpType
    ACT = mybir.ActivationFunctionType
    DR = mybir.MatmulPerfMode.DoubleRow

    KQ = 16 if COMP["qkv"] else 8
    KP = (16 if COMP["proj"] else 8) + (2 if bias_p else 0)
    K1 = 16 if COMP["fc1"] else 8
    K2 = (64 if COMP["fc2"] else 32) + (2 if bias_f else 0)
    NCH_A = 8 + (2 if bias_p else 0)   # attnT chunks (+ones pair)
    NCH_H = 32 + (2 if bias_f else 0)  # hT chunks (+ones pair)

    nc = bacc.Bacc()

    xmT_d = nc.declare_dram_parameter("xmT", [P, 8 * TH], bf16,
                                      isOutput=False)
    qkvw_ds = [nc.declare_dram_parameter(f"qkvw{b}", [P, KQ * 768], fp8,
                                         isOutput=False) for b in range(4)]
    projw_d = nc.declare_dram_parameter("projw", [P, KP * D], fp8,
                                        isOutput=False)
    fc1w_ds = [nc.declare_dram_parameter(f"fc1w{b}", [P, K1 * 2048], fp8,
                                         isOutput=False) for b in range(2)]
    fc2w_ds = [nc.declare_dram_parameter(f"fc2w{b}", [P, K2 * 512], fp8,
                                         isOutput=False) for b in range(2)]
    cpak_d = nc.declare_dram_parameter("cpak", [P, CPAK_W], f32,
                                       isOutput=False)
    bpak_d = nc.declare_dram_parameter("bpak", [P, BPAK_W], bf16,
                                       isOutput=False)
    out_d = nc.declare_dram_parameter("out", [D, T], bf16, isOutput=True)
    # DRAM scratch for the probs partition-broadcast round-trip
    pw_dram = nc.dram_tensor("pwd", (H, 3 * T), bf16, kind="Internal")
    rz_dram = nc.dram_tensor("rzd", (H, T), bf16, kind="Internal")

    with tile.TileContext(nc) as tc, ExitStack() as ctx:
        # PSUM budget (8 banks): mm x4, sc x2, tp x2
        const = ctx.enter_context(tc.tile_pool(name="const", bufs=1))
        acts = ctx.enter_context(tc.tile_pool(name="acts", bufs=1))
        ln_pool = ctx.enter_context(tc.tile_pool(name="ln", bufs=2))
        tp_ps = ctx.enter_context(tc.tile_pool(name="tp_ps", bufs=2,
                                               space="PSUM"))
        mm_ps = ctx.enter_context(tc.tile_pool(name="mm_ps", bufs=4,
                                               space="PSUM"))
        sc_ps = ctx.enter_context(tc.tile_pool(name="sc_ps", bufs=2,
                                               space="PSUM"))

        bpak = const.tile([P, BPAK_W], bf16, tag="bp", name="bpak")
        cpak = const.tile([P, CPAK_W], f32, tag="cp", name="cpak")

        def cp(name, j, w=1):
            o = _C[name] + j
            return cpak[:, o:o + w]

        hmask = bpak[:, _B["hmask"]:_B["hmask"] + 128]
        emk = bpak[0:H, _B["emk"]:_B["emk"] + 3]
        em2 = bpak[0:H, _B["em2"]:_B["em2"] + 128]

        _wn = [0]

        def warm_mm(mov_ap):
            # ~2ns matmul keyed on a just-produced tile: keeps the PE
            # p-state ramp alive through engine-bound phases
            wp = mm_ps.tile([P, T], f32, tag="mm", name=f"wm{_wn[0]}")
            _wn[0] += 1
            nc.tensor.matmul(wp[:, 0:4], hmask[:, 0:128], mov_ap,
                             start=True, stop=True)

        # activations alive into the MLP phases (channel-major residual)
        x2T = acts.tile([P, 8, T], bf16, tag="x2T", name="x2T")
        x2lnT = acts.tile([P, 8, T], fp8, tag="x2lnT", name="x2lnT")
        # weights preloaded early so their DMAs overlap earlier phases
        projw = acts.tile([P, KP, D], fp8, tag="projw", name="projw")
        fc1w = [acts.tile([P, K1, 2048], fp8, tag=f"fc1w{b}",
                          name=f"fc1w{b}") for b in range(2)]
        fc2w = [acts.tile([P, K2, 512], fp8, tag=f"fc2w{b}",
                          name=f"fc2w{b}") for b in range(2)]

        def ln_tiles(pool, pfx, ntok):
            def t_(shape, tag):
                return pool.tile(shape, bf16, tag=pfx + tag,
                                 name=pfx + tag)
            return {
                "sq": t_([P, 8, ntok], "sq"),   # also reused as t1 space
                "f1": t_([P, 4, ntok], "f1"),
                "gq1": t_([P, 4, ntok], "gq1"),
                "ars": t_([P, ntok], "ars"),
                "arq": t_([P, ntok], "arq"),
                "mu": t_([P, ntok], "mu"),
                "e2n": t_([P, ntok], "e2n"),
                "mu2": t_([P, ntok], "mu2"),
                "stdt": t_([P, ntok], "stdt"),
            }

        def ln_chan(src, ntok, dst, pool, pfx, tiles=None,
                    skip_fold1=False, warm=None):
            """Channel-major layernorm: src [P, 8, ntok] bf16 ->
            dst [P, 8, ntok] fp8, per-token stats over the 1024 channels.
            Pairwise chunk folds (DVE sum-path, Pool square-path), gpsimd
            partition all-reduce broadcasts the per-token sums, bf16 row
            math, two-op apply; all pipelined in two token halves."""
            tl = tiles or ln_tiles(pool, pfx, ntok)
            sq, f1, gq1 = tl["sq"], tl["f1"], tl["gq1"]
            ars, arq, mu = tl["ars"], tl["arq"], tl["mu"]
            e2n, mu2, stdt = tl["e2n"], tl["mu2"], tl["stdt"]
            nh = ntok // 2
            for h2 in range(2):
                s = slice(h2 * nh, (h2 + 1) * nh) if h2 < 1 \
                    else slice(nh, ntok)
                # alternate the sum/square fold paths between DVE and
                # Pool per half so the halves pipeline across engines
                ev = nc.vector if h2 % 2 == 0 else nc.gpsimd
                eg = nc.gpsimd if h2 % 2 == 0 else nc.vector
                if not skip_fold1:
                    nc.vector.tensor_mul(sq[:, :, s], src[:, :, s],
                                         src[:, :, s])
                    ev.tensor_add(f1[:, :, s], src[:, 0:4, s],
                                  src[:, 4:8, s])
                    eg.tensor_add(gq1[:, :, s], sq[:, 0:4, s],
                                  sq[:, 4:8, s])
                ev.tensor_add(f1[:, 0:2, s], f1[:, 0:2, s], f1[:, 2:4, s])
                eg.tensor_add(gq1[:, 0:2, s], gq1[:, 0:2, s],
                              gq1[:, 2:4, s])
                ev.tensor_add(f1[:, 0, s], f1[:, 0, s], f1[:, 1, s])
                eg.tensor_add(gq1[:, 0, s], gq1[:, 0, s], gq1[:, 1, s])
                nc.gpsimd.partition_all_reduce(ars[:, s], f1[:, 0, s], P,
                                               bass_isa.ReduceOp.add)
                nc.gpsimd.partition_all_reduce(arq[:, s], gq1[:, 0, s], P,
                                               bass_isa.ReduceOp.add)
                if warm is not None:
                    warm(f1[:, 0, s.start:s.start + 4])
                ev.tensor_scalar_mul(mu[:, s], ars[:, s], 1.0 / D)
                ev.tensor_scalar(e2n[:, s], arq[:, s], 1.0 / D,
                                 EPS, ALU.mult, ALU.add)
                eg.tensor_mul(mu2[:, s], mu[:, s], mu[:, s])
                ev.tensor_sub(e2n[:, s], e2n[:, s], mu2[:, s])
                nc.scalar.activation(stdt[:, s], e2n[:, s], ACT.Sqrt)
                with nc.allow_low_precision(reason="ln bf16 rows"):
                    nc.vector.reciprocal(stdt[:, s], stdt[:, s])
                eg.tensor_mul(mu[:, s], mu[:, s], stdt[:, s])
                if warm is not None:
                    warm(stdt[:, s.start:s.start + 4])
                # stdt now holds rstd; mu holds mu*rstd
                for ch in range(8):
                    ea = ev if ch % 2 == 0 else eg
                    eb = eg if ch % 2 == 0 else ev
                    ea.tensor_mul(sq[:, ch, s], src[:, ch, s], stdt[:, s])
                    eb.tensor_sub(dst[:, ch, s], sq[:, ch, s], mu[:, s])
                    if warm is not None and ch % 3 == 2:
                        warm(dst[:, ch, s.start:s.start + 4])

        with tc.tile_pool(name="p1", bufs=1) as p1:
            xmT = p1.tile([P, 8, TH], bf16, tag="xmT", name="xmT")
            xlnT = p1.tile([P, 8, TH], fp8, tag="xlnT", name="xlnT")
            qT = p1.tile([P, 8 * T], bf16, tag="qT", name="qT")
            kvT = p1.tile([P, 16, TH], bf16, tag="kvT", name="kvT")
            xT = xmT[:, :, 2:TH]  # residual view (real tokens)

            # SP queue: x first (LN1 critical), consts, q-half of qkv
            # weights, then projw/fc1w/fc2w. Act queue: k/v-half of qkv.
            xmT_dv = xmT_d[:].rearrange("p (c t) -> p c t", c=8)
            nc.sync.dma_start(xmT[:, :, 0:TH // 2],
                              xmT_dv[:, :, 0:TH // 2])
            nc.sync.dma_start(xmT[:, :, TH // 2:TH],
                              xmT_dv[:, :, TH // 2:TH])
            nc.sync.dma_start(bpak[:], bpak_d[:])
            nc.sync.dma_start(cpak[:], cpak_d[:])

            with tc.tile_pool(name="p3", bufs=1) as p3:
                attnT = p3.tile([P, NCH_A, T], fp8, tag="attnT",
                                name="attnT")
                if bias_p:
                    nc.vector.memset(attnT[:, 8, :], 1.0)
                    nc.vector.memzero(attnT[:, 9, :])
                with tc.tile_pool(name="p2", bufs=1) as p2:
                    et = p2.tile([H, 3, T], bf16, tag="et", name="et")
                    with tc.tile_pool(name="wq", bufs=1) as wq_pool:
                        qkvw = []
                        for b in range(4):
                            t = wq_pool.tile([P, KQ, 768], fp8,
                                             tag=f"qkvw{b}", name=f"qkvw{b}")
                            eng = nc.sync if b < 2 else nc.scalar
                            eng.dma_start(t[:], qkvw_ds[b][:])
                            qkvw.append(t)
                        nc.sync.dma_start(projw[:], projw_d[:])
                        for b in range(2):
                            nc.sync.dma_start(fc1w[b][:], fc1w_ds[b][:])
                        for b in range(2):
                            nc.sync.dma_start(fc2w[b][:], fc2w_ds[b][:])

                        # ---- LN1 (channel-major, incl. halo columns) ----
                        with tc.tile_pool(name="lnp", bufs=1) as lnp:
                            ln_chan(xmT[:, :, :], TH, xlnT, lnp, "a")

                        # ---- QKV ----
                        # halo k/v columns: one psum tile = 16 blocks x 2
                        ph = tp_ps.tile([P, 8, 4], f32, tag="tp", name="ph")
                        for j in range(16):
                            col = D + j * P
                            wt = qkvw[col // 768]
                            wo = col % 768
                            for i in range(KQ // 2):
                                xc = (2 * i) % 8
                                nc.tensor.matmul(
                                    ph[:, j // 2, (j % 2) * 2:(j % 2) * 2 + 2],
                                    wt[:, 2 * i:2 * i + 2, wo:wo + P],
                                    xlnT[:, xc:xc + 2, 0:2],
                                    start=(i == 0), stop=(i == KQ // 2 - 1),
                                    perf_mode=DR,
                                )
                        pht = ln_pool.tile([P, 32], f32, tag="pht",
                                           name="pht")
                        nc.vector.tensor_mul(pht[:], ph[:, :, :],
                                             cp("khs", 0, 32))
                        nc.gpsimd.tensor_add(
                            kvT[:, :, 0:2], pht[:],
                            cp("khb", 0, 32))

                        def qkv_tile(j):
                            wt = qkvw[j // 6]
                            wo = (j % 6) * P
                            ps = mm_ps.tile([P, T], f32, tag="mm",
                                            name=f"qkv{j}")
                            for i in range(KQ // 2):
                                xc = (2 * i) % 8
                                nc.tensor.matmul(
                                    ps[:], wt[:, 2 * i:2 * i + 2, wo:wo + P],
                                    xlnT[:, xc:xc + 2, 2:TH],
                                    start=(i == 0), stop=(i == KQ // 2 - 1),
                                    perf_mode=DR,
                                )
                            if j < 8:
                                dst = qT[:, j * T:(j + 1) * T]
                            else:
                                dst = kvT[:, j - 8, 2:TH]
                            if j % 2 == 0:
                                nc.vector.tensor_scalar(
                                    dst, ps[:], cp("qkvs", j), cp("qkvb", j),
                                    ALU.mult, ALU.add)
                            else:
                                nc.scalar.activation(dst, ps[:], ACT.Identity,
                                                     bias=cp("qkvb", j),
                                                     scale=cp("qkvs", j))

                        for j in range(16):      # q then k
                            qkv_tile(j)
                        # Per-window: e-muls (DVE/Pool), score matmul, exp,
                        # then IMMEDIATE broadcast of the unnormalized
                        # probs (SBUF -> DRAM -> stride-0 fan-out; channels
                        # are head-minor so head = partition//8 in every
                        # chunk). The softmax 1/z is broadcast the same way
                        # and folded into the AV tail.
                        bcs = p2.tile([P, 3, T], bf16, tag="bcs", name="bcs")
                        for w in range(3):
                            e = p2.tile([P, 4, T], bf16, tag="e", bufs=2,
                                        name=f"e{w}")
                            e2 = p2.tile([P, 4, T], bf16, tag="e", bufs=2,
                                         name=f"e2{w}")
                            nc.vector.tensor_mul(
                                e[:], qT[:, 0:4 * T],
                                kvT[:, 0:4, 2 - w:2 - w + T])
                            e2eng = nc.gpsimd if w < 2 else nc.vector
                            e2eng.tensor_mul(
                                e2[:], qT[:, 4 * T:8 * T],
                                kvT[:, 4:8, 2 - w:2 - w + T])
                            sc = sc_ps.tile([H, T], f32, tag="sc",
                                            name=f"sc{w}")
                            for ch in range(8):
                                esrc = e if ch < 4 else e2
                                nc.tensor.matmul(
                                    sc[:], hmask[:, ch * H:(ch + 1) * H],
                                    esrc[:, ch % 4, :],
                                    start=(ch == 0), stop=(ch == 7),
                                )
                            nc.scalar.activation(et[:, w, :], sc[:], ACT.Exp)
                            if w == 1:
                                nc.gpsimd.tensor_mul(et[:, 1, 0:1],
                                                     et[:, 1, 0:1],
                                                     emk[:, 0:1])
                            elif w == 2:
                                nc.gpsimd.tensor_mul(et[:, 2, 0:2],
                                                     et[:, 2, 0:2],
                                                     emk[:, 1:3])

                        # preload the sqrt act table for LN2 while Act
                        # has slack (identity is in every set)
                        scr = ln_pool.tile([P, 1], f32, tag="scr", name="scr")
                        nc.scalar.activation(scr[:], cp("qkvs", 0), ACT.Sqrt)
                        # ---- softmax normalizer ----
                        z0 = p2.tile([H, T], bf16, tag="z0", name="z0")
                        z1 = p2.tile([H, T], bf16, tag="z1", name="z1")
                        rz = p2.tile([H, T], bf16, tag="z0", name="rz")
                        nc.gpsimd.tensor_add(z0[:], et[:, 0, :], et[:, 1, :])
                        nc.gpsimd.tensor_add(z1[:], z0[:], et[:, 2, :])
                        with nc.allow_low_precision(reason="softmax bf16"):
                            nc.vector.reciprocal(rz[:], z1[:])
                        nc.gpsimd.dma_start(rz_dram[:], rz[:])
                        rzb = p2.tile([P, T], bf16, tag="rzb", name="rzb")
                        nc.gpsimd.dma_start(
                            rzb[:], cap.AP(rz_dram, 0,
                                           [[T, H], [0, 8], [1, T]]))
                        for j in range(16, 24):  # v
                            qkv_tile(j)
                        # head->partition broadcasts on the now-idle PE
                        for w in range(3):
                            bcp = mm_ps.tile([P, T], f32, tag="mm",
                                             name=f"bcp{w}")
                            nc.tensor.matmul(bcp[:], em2[:, :], et[:, w, :],
                                             start=True, stop=True)
                            if w == 1:
                                nc.vector.tensor_copy(bcs[:, w, :], bcp[:])
                            else:
                                nc.scalar.activation(bcs[:, w, :], bcp[:],
                                                     ACT.Identity)


                        rzb2 = rzb[:].unsqueeze(1).broadcast_to([P, 2, T])
                        for chp in range(4):  # chunk pairs, fully streamed
                            ch = 2 * chp
                            avs = []
                            for w in range(3):
                                av = p2.tile([P, 2, T], bf16, tag="av",
                                             bufs=6, name=f"av{chp}_{w}")
                                nc.vector.tensor_mul(
                                    av[:],
                                    bcs[:, w, :].unsqueeze(1)
                                    .broadcast_to([P, 2, T]),
                                    kvT[:, 8 + ch:10 + ch,
                                        2 - w:2 - w + T],
                                )
                                avs.append(av)
                            av01 = p2.tile([P, 2, T], bf16, tag="av01",
                                           bufs=3, name=f"av01_{chp}")
                            eng = nc.vector if chp == 3 else nc.gpsimd
                            eng.tensor_add(av01[:], avs[0][:], avs[1][:])
                            eng.tensor_add(av01[:], av01[:], avs[2][:])
                            eng2 = nc.gpsimd if chp % 2 == 0 else nc.vector
                            eng2.tensor_mul(attnT[:, ch:ch + 2, :],
                                            av01[:], rzb2)

                # ---- proj + residual 1 + LN2 (all channel-major) ----
                with tc.tile_pool(name="p5", bufs=1) as p5:
                    pjps = {}
                    for j in range(8):
                        pool, tag = [(sc_ps, "sc"), (mm_ps, "mm"),
                                     (tp_ps, "tp")][0 if j < 2 else
                                                    (1 if j < 6 else 2)]
                        pjps[j] = pool.tile([P, T], f32, tag=tag,
                                            name=f"pj{j}")
                    for i in range(KP // 2):
                        for j in range(8):
                            nc.tensor.matmul(
                                pjps[j][:], projw[:, 2 * i:2 * i + 2,
                                                  j * P:(j + 1) * P],
                                attnT[:, 2 * i:2 * i + 2, :],
                                start=(i == 0), stop=(i == KP // 2 - 1),
                                perf_mode=DR,
                            )
                    # fused evict + scale + residual: x2T = pj*s + x.
                    # Even groups via DVE STT; odd via Act evict + Pool add.
                    ytmp = p5.tile([P, 4, T], bf16, tag="ytmp", name="ytmp")
                    tl2 = ln_tiles(p5, "b", T)
                    for c in range(4):
                        for j in (c, c + 4):
                            if j % 2 == 0:
                                nc.vector.scalar_tensor_tensor(
                                    x2T[:, j, :], pjps[j][:],
                                    cp("projs", j),
                                    xT[:, j, :], ALU.mult, ALU.add)
                            else:
                                nc.scalar.activation(ytmp[:, j // 2, :],
                                                     pjps[j][:],
                                                     ACT.Identity,
                                                     scale=cp("projs", j))
                                nc.gpsimd.tensor_add(x2T[:, j, :],
                                                     ytmp[:, j // 2, :],
                                                     xT[:, j, :])
                    # ---- LN2 ----
                    ln_chan(x2T[:, :, :], T, x2lnT, p5, "b", tiles=tl2,
                            warm=warm_mm)
                    scr2 = ln_pool.tile([P, 1], f32, tag="scr", name="scr2")
                    nc.scalar.activation(scr2[:], cp("qkvs", 0), ACT.Gelu)

        # ---- MLP fc1 + gelu, fc2 + residual 2 + store ----
        # fc1 tiles rotate on tp_ps; fc2 keeps 6 psum groups live on
        # mm_ps+sc_ps for the whole phase, its i-step lagging the fc1
        # round that produced those hT chunks by one round so the
        # in-order PE queue never stalls on a gelu eviction.
        with tc.tile_pool(name="w1", bufs=1) as w1_pool:
                mT = w1_pool.tile([P, 8 * T], bf16, tag="mT", name="mT")
                hT = w1_pool.tile([P, NCH_H, T], fp8, tag="hT", name="hT")
                if bias_f:
                    nc.vector.memset(hT[:, 32, :], 1.0)
                    nc.vector.memzero(hT[:, 33, :])

                def f2_mm(ps, j, i):
                    wt = fc2w[j // 4]
                    wo = (j % 4) * P
                    nc.tensor.matmul(
                        ps[:], wt[:, 2 * i:2 * i + 2, wo:wo + P],
                        hT[:, 2 * i:2 * i + 2, :],
                        start=(i == 0), stop=(i == K2 // 2 - 1),
                        perf_mode=DR,
                    )

                # fused evict + scale + residual, then store the chunk.
                # Even groups: DVE STT from psum; odd: Act evict + Pool add.
                mtmp = w1_pool.tile([P, 4, T], bf16, tag="mtmp", name="mtmp")

                def f2_evict(ps, j):
                    if j % 2 == 1:
                        nc.vector.scalar_tensor_tensor(
                            mT[:, j * T:(j + 1) * T], ps[:], cp("fc2s", j),
                            x2T[:, j, :], ALU.mult, ALU.add)
                    else:
                        nc.scalar.activation(mtmp[:, j // 2, :], ps[:],
                                             ACT.Identity,
                                             scale=cp("fc2s", j))
                        nc.gpsimd.tensor_add(mT[:, j * T:(j + 1) * T],
                                             mtmp[:, j // 2, :],
                                             x2T[:, j, :])
                    oq = nc.sync if j % 2 == 0 else nc.scalar
                    oq.dma_start(out_d[j * P:(j + 1) * P, :],
                                 mT[:, j * T:(j + 1) * T])

                f2ps = {}
                for j in range(6):
                    pool = sc_ps if j < 2 else mm_ps
                    f2ps[j] = pool.tile([P, T], f32,
                                        tag="sc" if j < 2 else "mm",
                                        name=f"f2{j}")

                for r in range(16):
                    for jj in (2 * r, 2 * r + 1):
                        wt = fc1w[jj // 16]
                        wo = (jj % 16) * P
                        ps = tp_ps.tile([P, T], f32, tag="tp", name=f"f1{jj}")
                        for i in range(K1 // 2):
                            xc = (2 * i) % 8
                            nc.tensor.matmul(
                                ps[:], wt[:, 2 * i:2 * i + 2, wo:wo + P],
                                x2lnT[:, xc:xc + 2, :],
                                start=(i == 0), stop=(i == K1 // 2 - 1),
                                perf_mode=DR,
                            )
                        nc.scalar.activation(hT[:, jj, :], ps[:], ACT.Gelu,
                                             bias=cp("fc1b", jj),
                                             scale=cp("fc1s", jj))
                    if r >= 1:
                        for j in range(6):
                            f2_mm(f2ps[j][:], j, r - 1)
                for j in range(6):
                    for i in range(15, K2 // 2):
                        f2_mm(f2ps[j][:], j, i)
                    f2_evict(f2ps[j][:], j)
                for j in (6, 7):
                    ps = tp_ps.tile([P, T], f32, tag="tp", name=f"f2{j}")
                    for i in range(K2 // 2):
                        f2_mm(ps[:], j, i)
                    f2_evict(ps[:], j)

    if not nc.is_finalized():
        nc.finalize()
    return nc


def _scale_w(w):
    amax = np.abs(w).max(axis=0, keepdims=True)
    s = 2.0 ** np.round(np.log2(2.0 / np.maximum(amax, 1e-30)))
    return w * s, (1.0 / s)[0]


def _prep_w(w, comp):
    """[Din, Dout] fp32 -> ([128, kchunks, Dout] fp8 chunk-major hi(+lo),
    descale vector [Dout])."""
    din, dout = w.shape
    nch = din // P
    ws, descale = _scale_w(np.ascontiguousarray(w.astype(np.float32)))
    hi = ws.astype(F8)
    blocks = [hi]
    if comp:
        lo = (ws - hi.astype(np.float32)).astype(F8)
        blocks.append(lo)
    cols = []
    for b in blocks:
        cols.append(b.reshape(nch, P, dout).transpose(1, 0, 2))
    out = np.concatenate(cols, axis=1)  # [128, kchunks, dout]
    return np.ascontiguousarray(out), descale.astype(np.float32)


def _perm():
    """Head-minor channel permutation: new channel k*128 + h*8 + j holds
    old channel h*64 + k*8 + j, so head(partition p) = p // 8 in every
    chunk of the transposed layout."""
    p = np.empty(D, np.int64)
    for k in range(8):
        for h in range(H):
            for j in range(8):
                p[k * P + h * 8 + j] = h * HD + k * 8 + j
    return p


def _host_inputs(x, qkv_w, qkv_b, proj_w, proj_b, g1, b1, g2, b2,
                 fc1_w, fc1_b, fc2_w, fc2_b):
    scale = HD ** -0.5
    qkvw_eff = (qkv_w * g1[:, None]).astype(np.float32).copy()
    qkvb_eff = (qkv_b + b1 @ qkv_w).astype(np.float32).copy()
    qkvw_eff[:, 0:D] *= scale
    qkvb_eff[0:D] *= scale
    pm = _perm()
    for s in range(3):
        qkvw_eff[:, s * D:(s + 1) * D] = qkvw_eff[:, s * D + pm]
        qkvb_eff[s * D:(s + 1) * D] = qkvb_eff[s * D + pm]
    proj_w = np.ascontiguousarray(proj_w[pm, :]).astype(np.float32)
    fc2_w = np.asarray(fc2_w, np.float32)
    bias_p = bool(np.any(proj_b))
    bias_f = bool(np.any(fc2_b))
    if bias_p:  # ones-chunk pair: extra moving chunk of 1s picks up b/128
        proj_w = np.vstack([proj_w, np.tile(proj_b[None, :] / P, (P, 1)),
                            np.zeros((P, D), np.float32)])
    if bias_f:
        fc2_w = np.vstack([fc2_w, np.tile(fc2_b[None, :] / P, (P, 1)),
                           np.zeros((P, D), np.float32)])
    fc1w_eff = (fc1_w * g2[:, None]).astype(np.float32)
    fc1b_eff = (fc1_b + b2 @ fc1_w).astype(np.float32)

    qkvw_p, qkvs_v = _prep_w(qkvw_eff, COMP["qkv"])
    projw_p, projs_v = _prep_w(proj_w, COMP["proj"])
    fc1w_p, fc1s_v = _prep_w(fc1w_eff, COMP["fc1"])
    fc2w_p, fc2s_v = _prep_w(fc2_w, COMP["fc2"])

    cpak = np.zeros((P, CPAK_W), np.float32)

    def setc(name, vec, n):
        cpak[:, _C[name]:_C[name] + n] = vec.reshape(n, P).T

    setc("qkvb", qkvb_eff, 24)
    setc("qkvs", qkvs_v, 24)
    setc("projs", projs_v, 8)
    setc("fc1b", fc1b_eff, 32)
    setc("fc1s", fc1s_v, 32)
    setc("fc2s", fc2s_v, 8)
    kv_s = qkvs_v[D:3 * D].reshape(16, P)
    kv_b = qkvb_eff[D:3 * D].reshape(16, P)
    for j in range(16):
        for c in range(2):
            cpak[:, _C["khs"] + 2 * j + c] = kv_s[j]
            cpak[:, _C["khb"] + 2 * j + c] = kv_b[j]

    bpak0 = np.zeros((P, BPAK_W), np.float32)
    hm = np.zeros((P, 8, H), np.float32)
    for c in range(P):
        for ch in range(8):
            hm[c, ch, c // 8] = 1.0
    bpak0[:, _B["hmask"]:_B["hmask"] + 128] = hm.reshape(P, 8 * H)
    for c in range(P):
        bpak0[c // 8, _B["em2"] + c] = 1.0

    common = {
        "projw": np.ascontiguousarray(projw_p.reshape(P, -1)),
        "cpak": cpak,
    }
    for b in range(4):
        common[f"qkvw{b}"] = np.ascontiguousarray(
            qkvw_p[:, :, b * 768:(b + 1) * 768].reshape(P, -1))
    for b in range(2):
        common[f"fc1w{b}"] = np.ascontiguousarray(
            fc1w_p[:, :, b * 2048:(b + 1) * 2048].reshape(P, -1))
    for b in range(2):
        common[f"fc2w{b}"] = np.ascontiguousarray(
            fc2w_p[:, :, b * 512:(b + 1) * 512].reshape(P, -1))

    in_maps = []
    for core in range(NCORE):
        b, q = divmod(core, 4)
        xa = np.zeros((TH, D), np.float32)
        xa[2:] = x[b, q * T:(q + 1) * T, :]
        bpak = bpak0.copy()
        if q > 0:
            xa[0:2] = x[b, q * T - 2:q * T, :]
            bpak[0:H, _B["emk"]:_B["emk"] + 3] = 1.0
        # channel-major with halo prepended: xmT[p, ch, t]
        xmT = np.ascontiguousarray(
            xa.T.reshape(8, P, TH).transpose(1, 0, 2)).astype(BF)
        m = dict(common)
        m["xmT"] = xmT.reshape(P, -1)
        m["bpak"] = bpak.astype(BF)
        in_maps.append(m)
    return in_maps


def kernel(**inputs) -> np.ndarray:
    from concourse.bass_utils import run_bass_kernel_spmd

    key = (bool(np.any(inputs["proj_b"])), bool(np.any(inputs["fc2_b"])))
    if key not in _CACHE:
        _CACHE[key] = _build_program(bias_p=key[0], bias_f=key[1])
    nc = _CACHE[key]
    in_maps = _host_inputs(**inputs)
    res = run_bass_kernel_spmd(nc, in_maps, list(range(NCORE)))
    outs = res.results
    full = np.zeros((2, 2048, D), np.float32)
    for core in range(NCORE):
        b, q = divmod(core, 4)
        full[b, q * T:(q + 1) * T, :] = outs[core]["out"].astype(
            np.float32).T
    return full


# revision 96
# speedup vs baseline: 1.0169x; 1.0169x over previous
"""Trainium2 Bass kernel: LocalCausalTransformerBlock (window-3 causal attention).

Sharding: 8-way sequence-parallel. B=2 x N=2048 = 4096 tokens -> 8 chunks of
512 tokens (4 chunks per batch row). Each core gets its 512 tokens plus a
2-token halo (the preceding tokens of the same sequence, prepended host-side)
so the window-3 causal attention needs no cross-core communication. Weights
are replicated.

Everything on-device is channel-major (channels on partitions, tokens on the
free axis): the host hands x pre-transposed with the halo prepended and
un-transposes the output, so the PE does only matmuls - no transposes at all.
Both layernorms run channel-major: pairwise chunk folds + a gpsimd partition
all-reduce produce per-token sums broadcast to every partition, the rstd/mu
row math runs on those broadcast tiles in bf16, and the apply is two
tensor-tensor passes (in two token halves so downstream consumers start
sooner). LN gammas fold into the following matmul's weights host-side.

The four big matmuls (qkv/proj/fc1/fc2) run in fp8e4m3 with DoubleRow perf
mode (0.5 cycles/row). Weights are pre-scaled per output column to a power of
two near absmax~2; the descale rides the evictions. The proj and fc2
evictions are scalar_tensor_tensor ops that fuse descale + residual add in
one instruction (nonzero proj/fc2 biases, if ever present, ride an extra
ones-chunk in the contraction). k/v channels are host-permuted head-minor
(head = partition // 8, identical in every chunk) so the softmax probs
broadcast from 16 head rows to 128 partitions is a single stride-0 DMA via a
DRAM round-trip. Softmax needs no max-subtraction (window-3 scores are
small). DMAs are spread over the SP/Activation/Pool queues, which the cost
model executes concurrently.
"""

import sys

for _p in ("/opt/trn_rl_repo",):
    if _p not in sys.path:
        sys.path.insert(0, _p)

import numpy as np
import ml_dtypes

P = 128
D = 1024
H = 16
HD = 64
H3 = 3 * D
HID = 4096
T = 512            # real tokens per core
TH = T + 2         # token axis with 2-token halo (halo stored first)
NCORE = 8
EPS = 1e-5
BF = ml_dtypes.bfloat16
F8 = ml_dtypes.float8_e4m3

# which weights carry the fp8 quantization residual (2x k-chunks)
COMP = {"qkv": False, "proj": False, "fc1": False, "fc2": False}

# packed f32 const columns
_C = {}
_off = 0
for _name, _w in [("qkvb", 24), ("qkvs", 24), ("projs", 8),
                  ("fc1b", 32), ("fc1s", 32), ("fc2s", 8),
                  ("khs", 32), ("khb", 32)]:
    _C[_name] = _off
    _off += _w
CPAK_W = _off
# packed bf16 const columns: hmask, emk
_B = {"hmask": 0, "emk": 128, "em2": 131}
BPAK_W = 259

_CACHE: dict = {}


def _build_program(bias_p=False, bias_f=False):
    """bias_p/bias_f: include ones-chunks in proj/fc2 matmuls to add a
    nonzero proj_b/fc2_b (the fused residual evictions have no other slot
    for them). Left off when the biases are zero."""
    import concourse.bass as bass
    import concourse.tile as tile
    import concourse.ap as cap
    from concourse import bacc, mybir, bass_isa
    from contextlib import ExitStack

    f32 = mybir.dt.float32
    bf16 = mybir.dt.bfloat16
    fp8 = mybir.dt.float8e4
    ALU = mybir.AluOpType
    ACT = mybir.ActivationFunctionType
    DR = mybir.MatmulPerfMode.DoubleRow

    KQ = 16 if COMP["qkv"] else 8
    KP = (16 if COMP["proj"] else 8) + (2 if bias_p else 0)
    K1 = 16 if COMP["fc1"] else 8
    K2 = (64 if COMP["fc2"] else 32) + (2 if bias_f else 0)
    NCH_A = 8 + (2 if bias_p else 0)   # attnT chunks (+ones pair)
    NCH_H = 32 + (2 if bias_f else 0)  # hT chunks (+ones pair)

    nc = bacc.Bacc()

    xmT_d = nc.declare_dram_parameter("xmT", [P, 8 * TH], bf16,
                                      isOutput=False)
    qkvw_ds = [nc.declare_dram_parameter(f"qkvw{b}", [P, KQ * 768], fp8,
                                         isOutput=False) for b in range(4)]
    projw_d = nc.declare_dram_parameter("projw", [P, KP * D], fp8,
                                        isOutput=False)
    fc1w_ds = [nc.declare_dram_parameter(f"fc1w{b}", [P, K1 * 2048], fp8,
                                         isOutput=False) for b in range(2)]
    fc2w_ds = [nc.declare_dram_parameter(f"fc2w{b}", [P, K2 * 512], fp8,
                                         isOutput=False) for b in range(2)]
    cpak_d = nc.declare_dram_parameter("cpak", [P, CPAK_W], f32,
                                       isOutput=False)
    bpak_d = nc.declare_dram_parameter("bpak", [P, BPAK_W], bf16,
                                       isOutput=False)
    out_d = nc.declare_dram_parameter("out", [D, T], bf16, isOutput=True)
    # DRAM scratch for the probs partition-broadcast round-trip
    pw_dram = nc.dram_tensor("pwd", (H, 3 * T), bf16, kind="Internal")
    rz_dram = nc.dram_tensor("rzd", (H, T), bf16, kind="Internal")

    with tile.TileContext(nc) as tc, ExitStack() as ctx:
        # PSUM budget (8 banks): mm x4, sc x2, tp x2
        const = ctx.enter_context(tc.tile_pool(name="const", bufs=1))
        acts = ctx.enter_context(tc.tile_pool(name="acts", bufs=1))
        ln_pool = ctx.enter_context(tc.tile_pool(name="ln", bufs=2))
        tp_ps = ctx.enter_context(tc.tile_pool(name="tp_ps", bufs=2,
                                               space="PSUM"))
        mm_ps = ctx.enter_context(tc.tile_pool(name="mm_ps", bufs=4,
                                               space="PSUM"))
        sc_ps = ctx.enter_context(tc.tile_pool(name="sc_ps", bufs=2,
                                               space="PSUM"))

        bpak = const.tile([P, BPAK_W], bf16, tag="bp", name="bpak")
        cpak = const.tile([P, CPAK_W], f32, tag="cp", name="cpak")

        def cp(name, j, w=1):
            o = _C[name] + j
            return cpak[:, o:o + w]

        hmask = bpak[:, _B["hmask"]:_B["hmask"] + 128]
        emk = bpak[0:H, _B["emk"]:_B["emk"] + 3]
        em2 = bpak[0:H, _B["em2"]:_B["em2"] + 128]

        _wn = [0]

        def warm_mm(mov_ap):
            # ~2ns matmul keyed on a just-produced tile: keeps the PE
            # p-state ramp alive through engine-bound phases
            wp = mm_ps.tile([P, T], f32, tag="mm", name=f"wm{_wn[0]}")
            _wn[0] += 1
            nc.tensor.matmul(wp[:, 0:4], hmask[:, 0:128], mov_ap,
                             start=True, stop=True)

        # activations alive into the MLP phases (channel-major residual)
        x2T = acts.tile([P, 8, T], bf16, tag="x2T", name="x2T")
        x2lnT = acts.tile([P, 8, T], fp8, tag="x2lnT", name="x2lnT")
        # weights preloaded early so their DMAs overlap earlier phases
        projw = acts.tile([P, KP, D], fp8, tag="projw", name="projw")
        fc1w = [acts.tile([P, K1, 2048], fp8, tag=f"fc1w{b}",
                          name=f"fc1w{b}") for b in range(2)]
        fc2w = [acts.tile([P, K2, 512], fp8, tag=f"fc2w{b}",
                          name=f"fc2w{b}") for b in range(2)]

        def ln_tiles(pool, pfx, ntok):
            def t_(shape, tag):
                return pool.tile(shape, bf16, tag=pfx + tag,
                                 name=pfx + tag)
            return {
                "sq": t_([P, 8, ntok], "sq"),   # also reused as t1 space
                "f1": t_([P, 4, ntok], "f1"),
                "gq1": t_([P, 4, ntok], "gq1"),
                "ars": t_([P, ntok], "ars"),
                "arq": t_([P, ntok], "arq"),
                "mu": t_([P, ntok], "mu"),
                "e2n": t_([P, ntok], "e2n"),
                "mu2": t_([P, ntok], "mu2"),
                "stdt": t_([P, ntok], "stdt"),
            }

        def ln_chan(src, ntok, dst, pool, pfx, tiles=None,
                    skip_fold1=False, warm=None):
            """Channel-major layernorm: src [P, 8, ntok] bf16 ->
            dst [P, 8, ntok] fp8, per-token stats over the 1024 channels.
            Pairwise chunk folds (DVE sum-path, Pool square-path), gpsimd
            partition all-reduce broadcasts the per-token sums, bf16 row
            math, two-op apply; all pipelined in two token halves."""
            tl = tiles or ln_tiles(pool, pfx, ntok)
            sq, f1, gq1 = tl["sq"], tl["f1"], tl["gq1"]
            ars, arq, mu = tl["ars"], tl["arq"], tl["mu"]
            e2n, mu2, stdt = tl["e2n"], tl["mu2"], tl["stdt"]
            nh = ntok // 2
            for h2 in range(2):
                s = slice(h2 * nh, (h2 + 1) * nh) if h2 < 1 \
                    else slice(nh, ntok)
                # alternate the sum/square fold paths between DVE and
                # Pool per half so the halves pipeline across engines
                ev = nc.vector if h2 % 2 == 0 else nc.gpsimd
                eg = nc.gpsimd if h2 % 2 == 0 else nc.vector
                if not skip_fold1:
                    nc.vector.tensor_mul(sq[:, :, s], src[:, :, s],
                                         src[:, :, s])
                    ev.tensor_add(f1[:, :, s], src[:, 0:4, s],
                                  src[:, 4:8, s])
                    eg.tensor_add(gq1[:, :, s], sq[:, 0:4, s],
                                  sq[:, 4:8, s])
                ev.tensor_add(f1[:, 0:2, s], f1[:, 0:2, s], f1[:, 2:4, s])
                eg.tensor_add(gq1[:, 0:2, s], gq1[:, 0:2, s],
                              gq1[:, 2:4, s])
                ev.tensor_add(f1[:, 0, s], f1[:, 0, s], f1[:, 1, s])
                eg.tensor_add(gq1[:, 0, s], gq1[:, 0, s], gq1[:, 1, s])
                nc.gpsimd.partition_all_reduce(ars[:, s], f1[:, 0, s], P,
                                               bass_isa.ReduceOp.add)
                nc.gpsimd.partition_all_reduce(arq[:, s], gq1[:, 0, s], P,
                                               bass_isa.ReduceOp.add)
                if warm is not None:
                    warm(f1[:, 0, s.start:s.start + 4])
                ev.tensor_scalar_mul(mu[:, s], ars[:, s], 1.0 / D)
                ev.tensor_scalar(e2n[:, s], arq[:, s], 1.0 / D,
                                 EPS, ALU.mult, ALU.add)
                eg.tensor_mul(mu2[:, s], mu[:, s], mu[:, s])
                ev.tensor_sub(e2n[:, s], e2n[:, s], mu2[:, s])
                nc.scalar.activation(stdt[:, s], e2n[:, s], ACT.Sqrt)
                with nc.allow_low_precision(reason="ln bf16 rows"):
                    nc.vector.reciprocal(stdt[:, s], stdt[:, s])
                eg.tensor_mul(mu[:, s], mu[:, s], stdt[:, s])
                if warm is not None:
                    warm(stdt[:, s.start:s.start + 4])
                # stdt now holds rstd; mu holds mu*rstd
                for ch in range(8):
                    ea = ev if ch % 2 == 0 else eg
                    eb = eg if ch % 2 == 0 else ev
                    ea.tensor_mul(sq[:, ch, s], src[:, ch, s], stdt[:, s])
                    eb.tensor_sub(dst[:, ch, s], sq[:, ch, s], mu[:, s])
                    if warm is not None and ch % 3 == 2:
                        warm(dst[:, ch, s.start:s.start + 4])

        with tc.tile_pool(name="p1", bufs=1) as p1:
            xmT = p1.tile([P, 8, TH], bf16, tag="xmT", name="xmT")
            xlnT = p1.tile([P, 8, TH], fp8, tag="xlnT", name="xlnT")
            qT = p1.tile([P, 8 * T], bf16, tag="qT", name="qT")
            kvT = p1.tile([P, 16, TH], bf16, tag="kvT", name="kvT")
            xT = xmT[:, :, 2:TH]  # residual view (real tokens)

            # SP queue: x first (LN1 critical), consts, q-half of qkv
            # weights, then projw/fc1w/fc2w. Act queue: k/v-half of qkv.
            xmT_dv = xmT_d[:].rearrange("p (c t) -> p c t", c=8)
            nc.sync.dma_start(xmT[:, :, 0:TH // 2],
                              xmT_dv[:, :, 0:TH // 2])
            nc.sync.dma_start(xmT[:, :, TH // 2:TH],
                              xmT_dv[:, :, TH // 2:TH])
            nc.sync.dma_start(bpak[:], bpak_d[:])
            nc.sync.dma_start(cpak[:], cpak_d[:])

            with tc.tile_pool(name="p3", bufs=1) as p3:
                attnT = p3.tile([P, NCH_A, T], fp8, tag="attnT",
                                name="attnT")
                if bias_p:
                    nc.vector.memset(attnT[:, 8, :], 1.0)
                    nc.vector.memzero(attnT[:, 9, :])
                with tc.tile_pool(name="p2", bufs=1) as p2:
                    et = p2.tile([H, 3, T], bf16, tag="et", name="et")
                    with tc.tile_pool(name="wq", bufs=1) as wq_pool:
                        qkvw = []
                        for b in range(4):
                            t = wq_pool.tile([P, KQ, 768], fp8,
                                             tag=f"qkvw{b}", name=f"qkvw{b}")
                            eng = nc.sync if b < 2 else nc.scalar
                            eng.dma_start(t[:], qkvw_ds[b][:])
                            qkvw.append(t)
                        nc.sync.dma_start(projw[:], projw_d[:])
                        for b in range(2):
                            nc.sync.dma_start(fc1w[b][:], fc1w_ds[b][:])
                        for b in range(2):
                            nc.sync.dma_start(fc2w[b][:], fc2w_ds[b][:])

                        # ---- LN1 (channel-major, incl. halo columns) ----
                        with tc.tile_pool(name="lnp", bufs=1) as lnp:
                            ln_chan(xmT[:, :, :], TH, xlnT, lnp, "a")

                        # ---- QKV ----
                        # halo k/v columns: one psum tile = 16 blocks x 2
                        ph = tp_ps.tile([P, 8, 4], f32, tag="tp", name="ph")
                        for j in range(16):
                            col = D + j * P
                            wt = qkvw[col // 768]
                            wo = col % 768
                            for i in range(KQ // 2):
                                xc = (2 * i) % 8
                                nc.tensor.matmul(
                                    ph[:, j // 2, (j % 2) * 2:(j % 2) * 2 + 2],
                                    wt[:, 2 * i:2 * i + 2, wo:wo + P],
                                    xlnT[:, xc:xc + 2, 0:2],
                                    start=(i == 0), stop=(i == KQ // 2 - 1),
                                    perf_mode=DR,
                                )
                        pht = ln_pool.tile([P, 32], f32, tag="pht",
                                           name="pht")
                        nc.vector.tensor_mul(pht[:], ph[:, :, :],
                                             cp("khs", 0, 32))
                        nc.gpsimd.tensor_add(
                            kvT[:, :, 0:2], pht[:],
                            cp("khb", 0, 32))

                        def qkv_tile(j):
                            wt = qkvw[j // 6]
                            wo = (j % 6) * P
                            ps = mm_ps.tile([P, T], f32, tag="mm",
                                            name=f"qkv{j}")
                            for i in range(KQ // 2):
                                xc = (2 * i) % 8
                                nc.tensor.matmul(
                                    ps[:], wt[:, 2 * i:2 * i + 2, wo:wo + P],
                                    xlnT[:, xc:xc + 2, 2:TH],
                                    start=(i == 0), stop=(i == KQ // 2 - 1),
                                    perf_mode=DR,
                                )
                            if j < 8:
                                dst = qT[:, j * T:(j + 1) * T]
                            else:
                                dst = kvT[:, j - 8, 2:TH]
                            if j % 2 == 0:
                                nc.vector.tensor_scalar(
                                    dst, ps[:], cp("qkvs", j), cp("qkvb", j),
                                    ALU.mult, ALU.add)
                            else:
                                nc.scalar.activation(dst, ps[:], ACT.Identity,
                                                     bias=cp("qkvb", j),
                                                     scale=cp("qkvs", j))

                        for j in range(16):      # q then k
                            qkv_tile(j)
                        # Per-window: e-muls (DVE/Pool), score matmul, exp,
                        # then IMMEDIATE broadcast of the unnormalized
                        # probs (SBUF -> DRAM -> stride-0 fan-out; channels
                        # are head-minor so head = partition//8 in every
                        # chunk). The softmax 1/z is broadcast the same way
                        # and folded into the AV tail.
                        bcs = p2.tile([P, 3, T], bf16, tag="bcs", name="bcs")
                        for w in range(3):
                            e = p2.tile([P, 4, T], bf16, tag="e", bufs=2,
                                        name=f"e{w}")
                            e2 = p2.tile([P, 4, T], bf16, tag="e", bufs=2,
                                         name=f"e2{w}")
                            nc.vector.tensor_mul(
                                e[:], qT[:, 0:4 * T],
                                kvT[:, 0:4, 2 - w:2 - w + T])
                            e2eng = nc.gpsimd if w < 2 else nc.vector
                            e2eng.tensor_mul(
                                e2[:], qT[:, 4 * T:8 * T],
                                kvT[:, 4:8, 2 - w:2 - w + T])
                            sc = sc_ps.tile([H, T], f32, tag="sc",
                                            name=f"sc{w}")
                            for ch in range(8):
                                esrc = e if ch < 4 else e2
                                nc.tensor.matmul(
                                    sc[:], hmask[:, ch * H:(ch + 1) * H],
                                    esrc[:, ch % 4, :],
                                    start=(ch == 0), stop=(ch == 7),
                                )
                            nc.scalar.activation(et[:, w, :], sc[:], ACT.Exp)
                            if w == 1:
                                nc.gpsimd.tensor_mul(et[:, 1, 0:1],
                                                     et[:, 1, 0:1],
                                                     emk[:, 0:1])
                            elif w == 2:
                                nc.gpsimd.tensor_mul(et[:, 2, 0:2],
                                                     et[:, 2, 0:2],
                                                     emk[:, 1:3])

                        # preload the sqrt act table for LN2 while Act
                        # has slack (identity is in every set)
                        scr = ln_pool.tile([P, 1], f32, tag="scr", name="scr")
                        nc.scalar.activation(scr[:], cp("qkvs", 0), ACT.Sqrt)
                        # ---- softmax normalizer ----
                        z0 = p2.tile([H, T], bf16, tag="z0", name="z0")
                        z1 = p2.tile([H, T], bf16, tag="z1", name="z1")
                        rz = p2.tile([H, T], bf16, tag="z0", name="rz")
                        nc.gpsimd.tensor_add(z0[:], et[:, 0, :], et[:, 1, :])
                        nc.gpsimd.tensor_add(z1[:], z0[:], et[:, 2, :])
                        with nc.allow_low_precision(reason="softmax bf16"):
                            nc.vector.reciprocal(rz[:], z1[:])
                        nc.gpsimd.dma_start(rz_dram[:], rz[:])
                        rzb = p2.tile([P, T], bf16, tag="rzb", name="rzb")
                        nc.gpsimd.dma_start(
                            rzb[:], cap.AP(rz_dram, 0,
                                           [[T, H], [0, 8], [1, T]]))
                        for j in range(16, 24):  # v
                            qkv_tile(j)
                        # head->partition broadcasts on the now-idle PE
                        for w in range(3):
                            bcp = mm_ps.tile([P, T], f32, tag="mm",
                                             name=f"bcp{w}")
                            nc.tensor.matmul(bcp[:], em2[:, :], et[:, w, :],
                                             start=True, stop=True)
                            if w == 1:
                                nc.vector.tensor_copy(bcs[:, w, :], bcp[:])
                            else:
                                nc.scalar.activation(bcs[:, w, :], bcp[:],
                                                     ACT.Identity)


                        rzb2 = rzb[:].unsqueeze(1).broadcast_to([P, 2, T])
                        for chp in range(4):  # chunk pairs, fully streamed
                            ch = 2 * chp
                            avs = []
                            for w in range(3):
                                av = p2.tile([P, 2, T], bf16, tag="av",
                                             bufs=4, name=f"av{chp}_{w}")
                                nc.vector.tensor_mul(
                                    av[:],
                                    bcs[:, w, :].unsqueeze(1)
                                    .broadcast_to([P, 2, T]),
                                    kvT[:, 8 + ch:10 + ch,
                                        2 - w:2 - w + T],
                                )
                                avs.append(av)
                            av01 = p2.tile([P, 2, T], bf16, tag="av01",
                                           bufs=2, name=f"av01_{chp}")
                            eng = nc.vector if chp == 3 else nc.gpsimd
                            eng.tensor_add(av01[:], avs[0][:], avs[1][:])
                            eng.tensor_add(av01[:], av01[:], avs[2][:])
                            eng2 = nc.gpsimd if chp % 2 == 0 else nc.vector
                            eng2.tensor_mul(attnT[:, ch:ch + 2, :],
                                            av01[:], rzb2)

                # ---- proj + residual 1 + LN2 (all channel-major) ----
                with tc.tile_pool(name="p5", bufs=1) as p5:
                    pjps = {}
                    for j in range(8):
                        pool, tag = [(sc_ps, "sc"), (mm_ps, "mm"),
                                     (tp_ps, "tp")][0 if j < 2 else
                                                    (1 if j < 6 else 2)]
                        pjps[j] = pool.tile([P, T], f32, tag=tag,
                                            name=f"pj{j}")
                    for i in range(KP // 2):
                        for j in range(8):
                            nc.tensor.matmul(
                                pjps[j][:], projw[:, 2 * i:2 * i + 2,
                                                  j * P:(j + 1) * P],
                                attnT[:, 2 * i:2 * i + 2, :],
                                start=(i == 0), stop=(i == KP // 2 - 1),
                                perf_mode=DR,
                            )
                    # fused evict + scale + residual: x2T = pj*s + x.
                    # Even groups via DVE STT; odd via Act evict + Pool add.
                    ytmp = p5.tile([P, 4, T], bf16, tag="ytmp", name="ytmp")
                    tl2 = ln_tiles(p5, "b", T)
                    for c in range(4):
                        for j in (c, c + 4):
                            if j % 2 == 0:
                                nc.vector.scalar_tensor_tensor(
                                    x2T[:, j, :], pjps[j][:],
                                    cp("projs", j),
                                    xT[:, j, :], ALU.mult, ALU.add)
                            else:
                                nc.scalar.activation(ytmp[:, j // 2, :],
                                                     pjps[j][:],
                                                     ACT.Identity,
                                                     scale=cp("projs", j))
                                nc.gpsimd.tensor_add(x2T[:, j, :],
                                                     ytmp[:, j // 2, :],
                                                     xT[:, j, :])
                    # ---- LN2 ----
                    ln_chan(x2T[:, :, :], T, x2lnT, p5, "b", tiles=tl2,
                            warm=warm_mm)
                    scr2 = ln_pool.tile([P, 1], f32, tag="scr", name="scr2")
                    nc.scalar.activation(scr2[:], cp("qkvs", 0), ACT.Gelu)

        # ---- MLP fc1 + gelu, fc2 + residual 2 + store ----
        # fc1 tiles rotate on tp_ps; fc2 keeps 6 psum groups live on
        # mm_ps+sc_ps for the whole phase, its i-step lagging the fc1
        # round that produced those hT chunks by one round so the
        # in-order PE queue never stalls on a gelu eviction.
        with tc.tile_pool(name="w1", bufs=1) as w1_pool:
                mT = w1_pool.tile([P, 8 * T], bf16, tag="mT", name="mT")
                hT = w1_pool.tile([P, NCH_H, T], fp8, tag="hT", name="hT")
                if bias_f:
                    nc.vector.memset(hT[:, 32, :], 1.0)
                    nc.vector.memzero(hT[:, 33, :])

                def f2_mm(ps, j, i):
                    wt = fc2w[j // 4]
                    wo = (j % 4) * P
                    nc.tensor.matmul(
                        ps[:], wt[:, 2 * i:2 * i + 2, wo:wo + P],
                        hT[:, 2 * i:2 * i + 2, :],
                        start=(i == 0), stop=(i == K2 // 2 - 1),
                        perf_mode=DR,
                    )

                # fused evict + scale + residual, then store the chunk.
                # Even groups: DVE STT from psum; odd: Act evict + Pool add.
                mtmp = w1_pool.tile([P, 4, T], bf16, tag="mtmp", name="mtmp")

                def f2_evict(ps, j):
                    if j % 2 == 1:
                        nc.vector.scalar_tensor_tensor(
                            mT[:, j * T:(j + 1) * T], ps[:], cp("fc2s", j),
                            x2T[:, j, :], ALU.mult, ALU.add)
                    else:
                        nc.scalar.activation(mtmp[:, j // 2, :], ps[:],
                                             ACT.Identity,
                                             scale=cp("fc2s", j))
                        nc.gpsimd.tensor_add(mT[:, j * T:(j + 1) * T],
                                             mtmp[:, j // 2, :],
                                             x2T[:, j, :])
                    oq = nc.sync if j % 2 == 0 else nc.scalar
                    oq.dma_start(out_d[j * P:(j + 1) * P, :],
                                 mT[:, j * T:(j + 1) * T])

                f2ps = {}
                for j in range(6):
                    pool = sc_ps if j < 2 else mm_ps
                    f2ps[j] = pool.tile([P, T], f32,
                                        tag="sc" if j < 2 else "mm",
                                        name=f"f2{j}")

                for r in range(16):
                    for jj in (2 * r, 2 * r + 1):
                        wt = fc1w[jj // 16]
                        wo = (jj % 16) * P
                        ps = tp_ps.tile([P, T], f32, tag="tp", name=f"f1{jj}")
                        for i in range(K1 // 2):
                            xc = (2 * i) % 8
                            nc.tensor.matmul(
                                ps[:], wt[:, 2 * i:2 * i + 2, wo:wo + P],
                                x2lnT[:, xc:xc + 2, :],
                                start=(i == 0), stop=(i == K1 // 2 - 1),
                                perf_mode=DR,
                            )
                        nc.scalar.activation(hT[:, jj, :], ps[:], ACT.Gelu,
                                             bias=cp("fc1b", jj),
                                             scale=cp("fc1s", jj))
                    if r >= 1:
                        for j in range(6):
                            f2_mm(f2ps[j][:], j, r - 1)
                for j in range(6):
                    for i in range(15, K2 // 2):
                        f2_mm(f2ps[j][:], j, i)
                    f2_evict(f2ps[j][:], j)
                for j in (6, 7):
                    ps = tp_ps.tile([P, T], f32, tag="tp", name=f"f2{j}")
                    for i in range(K2 // 2):
                        f2_mm(ps[:], j, i)
                    f2_evict(ps[:], j)

    if not nc.is_finalized():
        nc.finalize()
    return nc


def _scale_w(w):
    amax = np.abs(w).max(axis=0, keepdims=True)
    s = 2.0 ** np.round(np.log2(2.0 / np.maximum(amax, 1e-30)))
    return w * s, (1.0 / s)[0]


def _prep_w(w, comp):
    """[Din, Dout] fp32 -> ([128, kchunks, Dout] fp8 chunk-major hi(+lo),
    descale vector [Dout])."""
    din, dout = w.shape
    nch = din // P
    ws, descale = _scale_w(np.ascontiguousarray(w.astype(np.float32)))
    hi = ws.astype(F8)
    blocks = [hi]
    if comp:
        lo = (ws - hi.astype(np.float32)).astype(F8)
        blocks.append(lo)
    cols = []
    for b in blocks:
        cols.append(b.reshape(nch, P, dout).transpose(1, 0, 2))
    out = np.concatenate(cols, axis=1)  # [128, kchunks, dout]
    return np.ascontiguousarray(out), descale.astype(np.float32)


def _perm():
    """Head-minor channel permutation: new channel k*128 + h*8 + j holds
    old channel h*64 + k*8 + j, so head(partition p) = p // 8 in every
    chunk of the transposed layout."""
    p = np.empty(D, np.int64)
    for k in range(8):
        for h in range(H):
            for j in range(8):
                p[k * P + h * 8 + j] = h * HD + k * 8 + j
    return p


def _host_inputs(x, qkv_w, qkv_b, proj_w, proj_b, g1, b1, g2, b2,
                 fc1_w, fc1_b, fc2_w, fc2_b):
    scale = HD ** -0.5
    qkvw_eff = (qkv_w * g1[:, None]).astype(np.float32).copy()
    qkvb_eff = (qkv_b + b1 @ qkv_w).astype(np.float32).copy()
    qkvw_eff[:, 0:D] *= scale
    qkvb_eff[0:D] *= scale
    pm = _perm()
    for s in range(3):
        qkvw_eff[:, s * D:(s + 1) * D] = qkvw_eff[:, s * D + pm]
        qkvb_eff[s * D:(s + 1) * D] = qkvb_eff[s * D + pm]
    proj_w = np.ascontiguousarray(proj_w[pm, :]).astype(np.float32)
    fc2_w = np.asarray(fc2_w, np.float32)
    bias_p = bool(np.any(proj_b))
    bias_f = bool(np.any(fc2_b))
    if bias_p:  # ones-chunk pair: extra moving chunk of 1s picks up b/128
        proj_w = np.vstack([proj_w, np.tile(proj_b[None, :] / P, (P, 1)),
                            np.zeros((P, D), np.float32)])
    if bias_f:
        fc2_w = np.vstack([fc2_w, np.tile(fc2_b[None, :] / P, (P, 1)),
                           np.zeros((P, D), np.float32)])
    fc1w_eff = (fc1_w * g2[:, None]).astype(np.float32)
    fc1b_eff = (fc1_b + b2 @ fc1_w).astype(np.float32)

    qkvw_p, qkvs_v = _prep_w(qkvw_eff, COMP["qkv"])
    projw_p, projs_v = _prep_w(proj_w, COMP["proj"])
    fc1w_p, fc1s_v = _prep_w(fc1w_eff, COMP["fc1"])
    fc2w_p, fc2s_v = _prep_w(fc2_w, COMP["fc2"])

    cpak = np.zeros((P, CPAK_W), np.float32)

    def setc(name, vec, n):
        cpak[:, _C[name]:_C[name] + n] = vec.reshape(n, P).T

    setc("qkvb", qkvb_eff, 24)
    setc("qkvs", qkvs_v, 24)
    setc("projs", projs_v, 8)
    setc("fc1b", fc1b_eff, 32)
    setc("fc1s", fc1s_v, 32)
    setc("fc2s", fc2s_v, 8)
    kv_s = qkvs_v[D:3 * D].reshape(16, P)
    kv_b = qkvb_eff[D:3 * D].reshape(16, P)
    for j in range(16):
        for c in range(2):
            cpak[:, _C["khs"] + 2 * j + c] = kv_s[j]
            cpak[:, _C["khb"] + 2 * j + c] = kv_b[j]

    bpak0 = np.zeros((P, BPAK_W), np.float32)
    hm = np.zeros((P, 8, H), np.float32)
    for c in range(P):
        for ch in range(8):
            hm[c, ch, c // 8] = 1.0
    bpak0[:, _B["hmask"]:_B["hmask"] + 128] = hm.reshape(P, 8 * H)
    for c in range(P):
        bpak0[c // 8, _B["em2"] + c] = 1.0

    common = {
        "projw": np.ascontiguousarray(projw_p.reshape(P, -1)),
        "cpak": cpak,
    }
    for b in range(4):
        common[f"qkvw{b}"] = np.ascontiguousarray(
            qkvw_p[:, :, b * 768:(b + 1) * 768].reshape(P, -1))
    for b in range(2):
        common[f"fc1w{b}"] = np.ascontiguousarray(
            fc1w_p[:, :, b * 2048:(b + 1) * 2048].reshape(P, -1))
    for b in range(2):
        common[f"fc2w{b}"] = np.ascontiguousarray(
            fc2w_p[:, :, b * 512:(b + 1) * 512].reshape(P, -1))

    in_maps = []
    for core in range(NCORE):
        b, q = divmod(core, 4)
        xa = np.zeros((TH, D), np.float32)
        xa[2:] = x[b, q * T:(q + 1) * T, :]
        bpak = bpak0.copy()
        if q > 0:
            xa[0:2] = x[b, q * T - 2:q * T, :]
            bpak[0:H, _B["emk"]:_B["emk"] + 3] = 1.0
        # channel-major with halo prepended: xmT[p, ch, t]
        xmT = np.ascontiguousarray(
            xa.T.reshape(8, P, TH).transpose(1, 0, 2)).astype(BF)
        m = dict(common)
        m["xmT"] = xmT.reshape(P, -1)
        m["bpak"] = bpak.astype(BF)
        in_maps.append(m)
    return in_maps


def kernel(**inputs) -> np.ndarray:
    from concourse.bass_utils import run_bass_kernel_spmd

    key = (bool(np.any(inputs["proj_b"])), bool(np.any(inputs["fc2_b"])))
    if key not in _CACHE:
        _CACHE[key] = _build_program(bias_p=key[0], bias_f=key[1])
    nc = _CACHE[key]
    in_maps = _host_inputs(**inputs)
    res = run_bass_kernel_spmd(nc, in_maps, list(range(NCORE)))
    outs = res.results
    full = np.zeros((2, 2048, D), np.float32)
    for core in range(NCORE):
        b, q = divmod(core, 4)
        full[b, q * T:(q + 1) * T, :] = outs[core]["out"].astype(
            np.float32).T
    return full
